# Initial kernel scaffold
#
"""Tensor-parallel GQA attention prefill (B=1, S=2048, D=4096, 32 q-heads /
8 kv-heads, RoPE, causal) for 8 Trainium2 NeuronCores.

Sharding: head-parallel. Core g owns q-heads 4g..4g+3 and kv-head g
(exact GQA group), computes Q/K/V projections for its heads, RoPE,
causal attention, and the partial output projection over its 512
contraction dims of wo. The host sums the 8 partial outputs.

Per-core kernel (Bass/Tile):
  phase 1  Q/K/V projections from a resident transposed activation
           (T-layout [head_dim, seq]); RoPE applied as
           rot = cos2*qk + sin2*(J @ qk) with the pair-swap J done on
           the tensor engine; V transposed back to natural layout on PE.
  phase 2  attention computed transposed: scoresT[k,q] tiles via one
           matmul each (HD=128 contraction). Causality is structural:
           future k-tiles are skipped, partially-masked k-tiles restrict
           the matmul/exp to the live q range, and only the 128-wide
           block diagonal gets an additive -1e9 mask. exp on the scalar
           engine straight out of PSUM. Unnormalized attnV accumulates
           in PSUM; the softmax denominators are accumulated by a
           matmul with an all-ones stationary, which lands them
           partition-broadcast in PSUM so normalization is one
           approx-reciprocal + one multiply fused into the eviction.
  phase 3  output projection per 128-row chunk over 8 concurrent PSUM
           banks.

All matmuls run in bf16 with fp32 PSUM accumulation (fp32 matmul is 4x
slower on TRN2's PE).
"""

import sys

if "/opt/trn_rl_repo" not in sys.path:
    sys.path.insert(0, "/opt/trn_rl_repo")

from contextlib import ExitStack

import numpy as np
import ml_dtypes

import concourse.bass as bass
import concourse.tile as tile
from concourse import mybir, bacc

BF16 = mybir.dt.bfloat16
F32 = mybir.dt.float32
NBF = ml_dtypes.bfloat16

S = 2048
D = 4096
HD = 128
HQ = 4                      # q heads per core
N_CORES = 8
SCALE = 1.0 / float(np.sqrt(128.0))
NEG = -1e9


def build_nc(S=S, D=D, num_devices=N_CORES):
    NCT = D // 128          # contraction tiles over model dim
    NSB = S // 512          # 512-wide seq blocks
    NST = S // 128          # 128-wide seq tiles
    NO = HQ + 1             # rotated o-tiles: 4 q heads + 1 k head
    NOV = NO + 1            # + v head
    NEB = D // 512          # output-proj e blocks
    NJT = HQ                # contraction j-tiles in output proj
    WCOLS = NCT * 128       # per-o weight row length

    nc = bacc.Bacc("TRN2", target_bir_lowering=False, debug=False,
                   num_devices=num_devices)
    xt_d = nc.dram_tensor("xt", [D, S], BF16, kind="ExternalInput")
    wt_d = nc.dram_tensor("wt", [NOV, 128, WCOLS], BF16, kind="ExternalInput")
    wot_d = nc.dram_tensor("wot", [NJT, 128, D], BF16, kind="ExternalInput")
    cos2_d = nc.dram_tensor("cos2", [128, S], F32, kind="ExternalInput")
    sin2_d = nc.dram_tensor("sin2", [128, S], F32, kind="ExternalInput")
    jt_d = nc.dram_tensor("jt", [128, 128], BF16, kind="ExternalInput")
    id_d = nc.dram_tensor("ident", [128, 128], BF16, kind="ExternalInput")
    mask_d = nc.dram_tensor("maskt", [128, 128], BF16, kind="ExternalInput")
    out_d = nc.dram_tensor("out", [S, D], BF16, kind="ExternalOutput")

    with tile.TileContext(nc) as tc, ExitStack() as outer:
        const = outer.enter_context(tc.tile_pool(name="const", bufs=1))
        qkp = outer.enter_context(tc.tile_pool(name="qkrot", bufs=1))
        vp = outer.enter_context(tc.tile_pool(name="vnat", bufs=1))

        jt_sb = const.tile([128, 128], BF16)
        id_sb = const.tile([128, 128], BF16)
        mask_sb = const.tile([128, 128], BF16)
        ones_sb = const.tile([128, 128], BF16)
        nc.sync.dma_start(out=jt_sb, in_=jt_d[:])
        nc.sync.dma_start(out=id_sb, in_=id_d[:])
        nc.sync.dma_start(out=mask_sb, in_=mask_d[:])
        nc.vector.memset(ones_sb, 1.0)

        # Rotated Q,K in T-layout: o-tile-major [o*S + s]; o 0..3 = q heads,
        # o 4 = k head.
        qk_rot = qkp.tile([128, NO * S], BF16)
        # V natural layout, t-tile-major: v_nat[t_local, tt*128 + d]
        v_nat = vp.tile([128, S], BF16)

        # ---------------- phase 1: projections + RoPE ----------------
        with ExitStack() as ph1:
            xtp = ph1.enter_context(tc.tile_pool(name="xtp", bufs=1))
            csp = ph1.enter_context(tc.tile_pool(name="cossin", bufs=1))
            wst = ph1.enter_context(tc.tile_pool(name="wstream", bufs=2))
            vts = ph1.enter_context(tc.tile_pool(name="vtsb", bufs=1))
            qts = ph1.enter_context(tc.tile_pool(name="qtmp", bufs=4))
            rtm = ph1.enter_context(tc.tile_pool(name="ropetmp", bufs=3))
            pps = ph1.enter_context(tc.tile_pool(name="projps", bufs=6, space="PSUM"))
            jps = ph1.enter_context(tc.tile_pool(name="jps", bufs=2, space="PSUM"))

            def load_w(o, nchunk=4):
                w = wst.tile([128, WCOLS], BF16, tag="wsb", name=f"wsb_{o}")
                qn = WCOLS // nchunk
                for qd in range(nchunk):
                    nc.sync.dma_start(out=w[:, qd * qn:(qd + 1) * qn],
                                      in_=wt_d[o, :, qd * qn:(qd + 1) * qn])
                return w

            # weights for o=0,1 and the RoPE tables go to the DMA queues
            # before the 16MB activation load so the PE can start early;
            # o=0 in 8 small chunks so its first c-tiles land soonest
            w_pre = [load_w(0, nchunk=8), load_w(1)]
            cos_sb = csp.tile([128, S], F32)
            sin_sb = csp.tile([128, S], F32)
            nc.sync.dma_start(out=cos_sb, in_=cos2_d[:])
            nc.sync.dma_start(out=sin_sb, in_=sin2_d[:])

            # xt loads go through the Scalar engine's HWDGE path: DMA
            # triggers serialize at ~650ns on their issuing engine, and SP
            # is busy firing the weight loads. The first two c-tiles are
            # split in half so the first matmul's operand lands early.
            xt_sb = xtp.tile([128, NCT * S], BF16)
            for c in range(NCT):
                if c < 2:
                    hS = S // 2
                    for hf in range(2):
                        nc.scalar.dma_start(
                            out=xt_sb[:, c * S + hf * hS: c * S + (hf + 1) * hS],
                            in_=xt_d[c * 128:(c + 1) * 128,
                                     hf * hS:(hf + 1) * hS])
                else:
                    nc.scalar.dma_start(out=xt_sb[:, c * S:(c + 1) * S],
                                        in_=xt_d[c * 128:(c + 1) * 128, :])
            vt_sb = vts.tile([128, S], BF16)

            for o in range(NOV):
                w_sb = w_pre[o] if o < 2 else load_w(o)
                psl = [pps.tile([128, 512], F32, tag="projps",
                                name=f"projps_{o}_{i}")
                       for i in range(NSB)]
                for c in range(NCT):
                    for sb in range(NSB):
                        nc.tensor.matmul(
                            psl[sb], w_sb[:, c * 128:(c + 1) * 128],
                            xt_sb[:, c * S + sb * 512: c * S + sb * 512 + 512],
                            start=(c == 0), stop=(c == NCT - 1))
                for sb in range(NSB):
                    if o < NO:
                        # RoPE: rot = cos2*qt + sin2*(J @ qt)
                        qt_sb = qts.tile([128, 512], BF16)
                        nc.scalar.activation(
                            out=qt_sb, in_=psl[sb],
                            func=mybir.ActivationFunctionType.Copy)
                        jp = jps.tile([128, 512], F32, tag="jps")
                        nc.tensor.matmul(jp, jt_sb, qt_sb, start=True, stop=True)
                        t1 = rtm.tile([128, 512], F32, tag="rt", name="t1")
                        nc.vector.tensor_mul(
                            t1, qt_sb, cos_sb[:, sb * 512:(sb + 1) * 512])
                        nc.vector.tensor_mul(
                            jp, jp, sin_sb[:, sb * 512:(sb + 1) * 512])
                        nc.vector.tensor_add(
                            qk_rot[:, o * S + sb * 512: o * S + sb * 512 + 512],
                            t1, jp)
                    else:
                        nc.scalar.activation(
                            out=vt_sb[:, sb * 512:(sb + 1) * 512], in_=psl[sb],
                            func=mybir.ActivationFunctionType.Copy)
            # V: T-layout -> natural via PE transpose
            for t in range(NST):
                tp = jps.tile([128, 128], BF16, tag="jps")
                nc.tensor.transpose(tp, vt_sb[:, t * 128:(t + 1) * 128], id_sb)
                nc.vector.tensor_copy(v_nat[:, t * 128:(t + 1) * 128], tp)

        # ---------------- phase 2: attention ----------------
        aotp = outer.enter_context(tc.tile_pool(name="aot", bufs=1))
        wotp = outer.enter_context(tc.tile_pool(name="wotsb", bufs=1))
        # aot[d, j*S + s] = head j attention out (normalized), T-layout
        aot = aotp.tile([128, NJT * S], BF16)
        wot_sb = wotp.tile([128, NJT * D], BF16)

        with ExitStack() as ph2:
            etp = ph2.enter_context(tc.tile_pool(name="expt", bufs=6))
            rbp = ph2.enter_context(tc.tile_pool(name="rbc", bufs=2))
            spsp = ph2.enter_context(tc.tile_pool(name="sps", bufs=4, space="PSUM"))
            outpp = ph2.enter_context(tc.tile_pool(name="outps", bufs=2, space="PSUM"))
            rpsp = ph2.enter_context(tc.tile_pool(name="rps", bufs=2, space="PSUM"))

            for j in range(NJT):
                for half in range(2):
                    hw_ = D // 2
                    nc.sync.dma_start(
                        out=wot_sb[:, j * D + half * hw_: j * D + (half + 1) * hw_],
                        in_=wot_d[j, :, half * hw_:(half + 1) * hw_])

            for jq in range(NSB):
                nk = 4 * jq + 4       # causal: k-tiles 0..4jq+3
                for h in range(HQ):
                    outps = outpp.tile([128, 512], F32, tag="outps")
                    rps = rpsp.tile([128, 512], F32, tag="rps")
                    for kt in range(nk):
                        delta = kt - 4 * jq
                        a = max(delta, 0) * 128   # live q range [a, 512)
                        sps = spsp.tile([128, 512], F32, tag="sps")
                        nc.tensor.matmul(
                            sps[:, a:],
                            qk_rot[:, HQ * S + kt * 128: HQ * S + (kt + 1) * 128],
                            qk_rot[:, h * S + jq * 512 + a: h * S + jq * 512 + 512],
                            start=True, stop=True)
                        if delta >= 0:
                            # block-diagonal subtile: additive causal mask
                            nc.vector.tensor_add(
                                sps[:, a:a + 128], sps[:, a:a + 128], mask_sb)
                        et = etp.tile([128, 512], BF16, tag="et")
                        nc.scalar.activation(
                            out=et[:, a:], in_=sps[:, a:],
                            func=mybir.ActivationFunctionType.Exp, scale=SCALE)
                        nc.tensor.matmul(
                            outps[:, a:], v_nat[:, kt * 128:(kt + 1) * 128],
                            et[:, a:],
                            start=(kt == 0), stop=(kt == nk - 1))
                        # all-ones stationary -> denominators land
                        # partition-broadcast: rps[m, q] = r[q] for every m
                        nc.tensor.matmul(
                            rps[:, a:], ones_sb, et[:, a:],
                            start=(kt == 0), stop=(kt == nk - 1))
                    rinv = rbp.tile([128, 512], F32, tag="rinv")
                    nc.vector.reciprocal_approx_fast(out=rinv, in_=rps)
                    nc.vector.tensor_mul(
                        aot[:, h * S + jq * 512: h * S + jq * 512 + 512],
                        outps, rinv)

        # ---------------- phase 3: output projection ----------------
        with ExitStack() as ph3:
            stg = ph3.enter_context(tc.tile_pool(name="stage", bufs=10))
            opsp = ph3.enter_context(tc.tile_pool(name="ops", bufs=8, space="PSUM"))

            for stc in range(NST):
                psl = [opsp.tile([128, 512], F32, tag="ops",
                                 name=f"ops_{stc}_{i}")
                       for i in range(NEB)]
                for j in range(NJT):
                    for eb in range(NEB):
                        nc.tensor.matmul(
                            psl[eb],
                            aot[:, j * S + stc * 128: j * S + (stc + 1) * 128],
                            wot_sb[:, j * D + eb * 512: j * D + eb * 512 + 512],
                            start=(j == 0), stop=(j == NJT - 1))
                for eb in range(NEB):
                    stage = stg.tile([128, 512], BF16, tag="stage")
                    nc.scalar.activation(
                        out=stage, in_=psl[eb],
                        func=mybir.ActivationFunctionType.Copy)
                    # last row-chunk: halve the store DMAs so the kernel
                    # tail is not one full 256KB transfer deep
                    nsp = 2 if stc == NST - 1 else 1
                    for sp in range(nsp):
                        w_ = 512 // nsp
                        nc.sync.dma_start(
                            out=out_d[stc * 128:(stc + 1) * 128,
                                      eb * 512 + sp * w_:
                                      eb * 512 + (sp + 1) * w_],
                            in_=stage[:, sp * w_:(sp + 1) * w_])

    nc.compile()
    return nc


# ---------------------------------------------------------------------------
# host-side prep


def make_consts(cos, sin):
    """cos/sin: [S, 64] f32 -> replicated T-layout + J + identity + diag mask."""
    cos2 = np.repeat(np.ascontiguousarray(cos.T), 2, axis=0).astype(np.float32)
    sin2 = np.repeat(np.ascontiguousarray(sin.T), 2, axis=0).astype(np.float32)
    J = np.zeros((128, 128), np.float32)
    for p in range(64):
        J[2 * p, 2 * p + 1] = -1.0
        J[2 * p + 1, 2 * p] = 1.0
    jt = np.ascontiguousarray(J.T).astype(NBF)
    ident = np.eye(128, dtype=NBF)
    k_idx = np.arange(128)[:, None]
    q_idx = np.arange(128)[None, :]
    maskt = np.where(q_idx >= k_idx, 0.0, NEG).astype(np.float32)  # [k, q]
    return cos2, sin2, jt, ident, maskt.astype(NBF)


def prep_all(x, wq, wk, wv, wo, cos, sin, n_cores=N_CORES):
    NCT = D // 128
    x2 = np.asarray(x, np.float32).reshape(S, D)
    xt = np.ascontiguousarray(x2.T).astype(NBF)
    wq = np.asarray(wq, np.float32)
    wk = np.asarray(wk, np.float32)
    wv = np.asarray(wv, np.float32)
    wo = np.asarray(wo, np.float32)
    cos2, sin2, jt, ident, maskt = make_consts(
        np.asarray(cos, np.float32), np.asarray(sin, np.float32))
    in_maps = []
    for g in range(n_cores):
        w_cat = np.concatenate(
            [wq[g * 512:(g + 1) * 512],
             wk[g * 128:(g + 1) * 128],
             wv[g * 128:(g + 1) * 128]], axis=0)          # [768, D]
        # wt[o, p, c*128 + f] = w_cat[o*128 + f, c*128 + p]
        wt = np.ascontiguousarray(
            w_cat.reshape(6, 128, NCT, 128).transpose(0, 3, 2, 1)
        ).reshape(6, 128, NCT * 128).astype(NBF)
        wot = np.ascontiguousarray(
            wo[:, g * 512:(g + 1) * 512].T).reshape(4, 128, D).astype(NBF)
        in_maps.append({
            "xt": xt, "wt": wt, "wot": wot, "cos2": cos2, "sin2": sin2,
            "jt": jt, "ident": ident, "maskt": maskt,
        })
    return in_maps


_NC_CACHE = None


def _get_nc():
    global _NC_CACHE
    if _NC_CACHE is None:
        _NC_CACHE = build_nc()
    return _NC_CACHE


def kernel(x, wq, wk, wv, wo, cos, sin, mask, start_pos):
    # mask is the standard causal mask (start_pos=0 prefill) — the kernel
    # applies causality structurally, so neither input is shipped.
    from concourse.bass_utils import run_bass_kernel_spmd

    nc = _get_nc()
    in_maps = prep_all(x, wq, wk, wv, wo, cos, sin)
    res = run_bass_kernel_spmd(nc, in_maps, core_ids=list(range(N_CORES)))
    acc = np.zeros((S, D), np.float32)
    for r in res.results:
        acc += r["out"].astype(np.float32)
    return acc.reshape(1, S, D)



# revision 1
# speedup vs baseline: 1.3902x; 1.3902x over previous
"""Tensor-parallel GQA attention prefill (B=1, S=2048, D=4096, 32 q-heads /
8 kv-heads, RoPE, causal) for 8 Trainium2 NeuronCores.

Sharding: head-parallel. Core g owns q-heads 4g..4g+3 and kv-head g
(exact GQA group), computes Q/K/V projections for its heads, RoPE,
causal attention, and the partial output projection over its 512
contraction dims of wo. The host sums the 8 partial outputs.

Per-core kernel (Bass/Tile):
  phase 1  Q/K/V projections from a resident transposed activation
           (T-layout [head_dim, seq]); RoPE applied as
           rot = cos2*qk + sin2*(J @ qk) with the pair-swap J done on
           the tensor engine; V transposed back to natural layout on PE.
  phase 2  attention computed transposed: scoresT[k,q] tiles via one
           matmul each (HD=128 contraction). Causality is structural:
           future k-tiles are skipped, partially-masked k-tiles restrict
           the matmul/exp to the live q range, and only the 128-wide
           block diagonal gets an additive -1e9 mask. exp on the scalar
           engine straight out of PSUM. Unnormalized attnV accumulates
           in PSUM; the softmax denominators are accumulated by a
           matmul with an all-ones stationary, which lands them
           partition-broadcast in PSUM so normalization is one
           approx-reciprocal + one multiply fused into the eviction.
  phase 3  output projection per 128-row chunk over 8 concurrent PSUM
           banks.

All matmuls run in bf16 with fp32 PSUM accumulation (fp32 matmul is 4x
slower on TRN2's PE).
"""

import sys

if "/opt/trn_rl_repo" not in sys.path:
    sys.path.insert(0, "/opt/trn_rl_repo")

from contextlib import ExitStack

import numpy as np
import ml_dtypes

import concourse.bass as bass
import concourse.tile as tile
from concourse import mybir, bacc

BF16 = mybir.dt.bfloat16
F32 = mybir.dt.float32
NBF = ml_dtypes.bfloat16

S = 2048
D = 4096
HD = 128
HQ = 4                      # q heads per core
N_CORES = 8
SCALE = 1.0 / float(np.sqrt(128.0))
NEG = -1e9


def build_nc(S=S, D=D, num_devices=N_CORES):
    NCT = D // 128          # contraction tiles over model dim
    NSB = S // 512          # 512-wide seq blocks
    NST = S // 128          # 128-wide seq tiles
    NO = HQ + 1             # rotated o-tiles: 4 q heads + 1 k head
    NOV = NO + 1            # + v head
    NEB = D // 512          # output-proj e blocks
    NJT = HQ                # contraction j-tiles in output proj
    WCOLS = NCT * 128       # per-o weight row length

    nc = bacc.Bacc("TRN2", target_bir_lowering=False, debug=False,
                   num_devices=num_devices)
    xt_d = nc.dram_tensor("xt", [D, S], BF16, kind="ExternalInput")
    wt_d = nc.dram_tensor("wt", [NOV, 128, WCOLS], BF16, kind="ExternalInput")
    wot_d = nc.dram_tensor("wot", [NJT, 128, D], BF16, kind="ExternalInput")
    cos2_d = nc.dram_tensor("cos2", [128, S], F32, kind="ExternalInput")
    sin2_d = nc.dram_tensor("sin2", [128, S], F32, kind="ExternalInput")
    jt_d = nc.dram_tensor("jt", [128, 128], BF16, kind="ExternalInput")
    id_d = nc.dram_tensor("ident", [128, 128], BF16, kind="ExternalInput")
    mask_d = nc.dram_tensor("maskt", [128, 128], BF16, kind="ExternalInput")
    out_d = nc.dram_tensor("out", [S, D], BF16, kind="ExternalOutput")

    with tile.TileContext(nc) as tc, ExitStack() as outer:
        const = outer.enter_context(tc.tile_pool(name="const", bufs=1))
        qkp = outer.enter_context(tc.tile_pool(name="qkrot", bufs=1))
        vp = outer.enter_context(tc.tile_pool(name="vnat", bufs=1))

        jt_sb = const.tile([128, 128], BF16)
        id_sb = const.tile([128, 128], BF16)
        mask_sb = const.tile([128, 128], BF16)
        ones_sb = const.tile([128, 128], BF16)
        nc.sync.dma_start(out=jt_sb, in_=jt_d[:])
        nc.sync.dma_start(out=id_sb, in_=id_d[:])
        nc.sync.dma_start(out=mask_sb, in_=mask_d[:])
        nc.vector.memset(ones_sb, 1.0)

        # Rotated Q,K in T-layout: o-tile-major [o*S + s]; o 0..3 = q heads,
        # o 4 = k head.
        qk_rot = qkp.tile([128, NO * S], BF16)
        # V natural layout, t-tile-major: v_nat[t_local, tt*128 + d]
        v_nat = vp.tile([128, S], BF16)

        # ---------------- phase 1: projections + RoPE ----------------
        with ExitStack() as ph1:
            xtp = ph1.enter_context(tc.tile_pool(name="xtp", bufs=1))
            csp = ph1.enter_context(tc.tile_pool(name="cossin", bufs=1))
            wst = ph1.enter_context(tc.tile_pool(name="wstream", bufs=2))
            vts = ph1.enter_context(tc.tile_pool(name="vtsb", bufs=1))
            qts = ph1.enter_context(tc.tile_pool(name="qtmp", bufs=4))
            rtm = ph1.enter_context(tc.tile_pool(name="ropetmp", bufs=3))
            pps = ph1.enter_context(tc.tile_pool(name="projps", bufs=6, space="PSUM"))
            jps = ph1.enter_context(tc.tile_pool(name="jps", bufs=2, space="PSUM"))

            def load_w(o, nchunk=4):
                w = wst.tile([128, WCOLS], BF16, tag="wsb", name=f"wsb_{o}")
                qn = WCOLS // nchunk
                for qd in range(nchunk):
                    nc.sync.dma_start(out=w[:, qd * qn:(qd + 1) * qn],
                                      in_=wt_d[o, :, qd * qn:(qd + 1) * qn])
                return w

            # weights for o=0,1 and the RoPE tables go to the DMA queues
            # before the 16MB activation load so the PE can start early;
            # o=0 in 8 small chunks so its first c-tiles land soonest
            w_pre = [load_w(0, nchunk=8), load_w(1)]
            cos_sb = csp.tile([128, S], F32)
            sin_sb = csp.tile([128, S], F32)
            nc.sync.dma_start(out=cos_sb, in_=cos2_d[:])
            nc.sync.dma_start(out=sin_sb, in_=sin2_d[:])

            # xt loads go through the Scalar engine's HWDGE path: DMA
            # triggers serialize at ~650ns on their issuing engine, and SP
            # is busy firing the weight loads. The first two c-tiles are
            # split in half so the first matmul's operand lands early.
            xt_sb = xtp.tile([128, NCT * S], BF16)
            for c in range(NCT):
                if c < 2:
                    hS = S // 2
                    for hf in range(2):
                        nc.scalar.dma_start(
                            out=xt_sb[:, c * S + hf * hS: c * S + (hf + 1) * hS],
                            in_=xt_d[c * 128:(c + 1) * 128,
                                     hf * hS:(hf + 1) * hS])
                else:
                    nc.scalar.dma_start(out=xt_sb[:, c * S:(c + 1) * S],
                                        in_=xt_d[c * 128:(c + 1) * 128, :])
            vt_sb = vts.tile([128, S], BF16)

            for o in range(NOV):
                w_sb = w_pre[o] if o < 2 else load_w(o)
                psl = [pps.tile([128, 512], F32, tag="projps",
                                name=f"projps_{o}_{i}")
                       for i in range(NSB)]
                for c in range(NCT):
                    for sb in range(NSB):
                        nc.tensor.matmul(
                            psl[sb], w_sb[:, c * 128:(c + 1) * 128],
                            xt_sb[:, c * S + sb * 512: c * S + sb * 512 + 512],
                            start=(c == 0), stop=(c == NCT - 1))
                for sb in range(NSB):
                    if o < NO:
                        # RoPE: rot = cos2*qt + sin2*(J @ qt)
                        qt_sb = qts.tile([128, 512], BF16)
                        nc.scalar.activation(
                            out=qt_sb, in_=psl[sb],
                            func=mybir.ActivationFunctionType.Copy)
                        jp = jps.tile([128, 512], F32, tag="jps")
                        nc.tensor.matmul(jp, jt_sb, qt_sb, start=True, stop=True)
                        t1 = rtm.tile([128, 512], F32, tag="rt", name="t1")
                        nc.vector.tensor_mul(
                            t1, qt_sb, cos_sb[:, sb * 512:(sb + 1) * 512])
                        nc.vector.tensor_mul(
                            jp, jp, sin_sb[:, sb * 512:(sb + 1) * 512])
                        nc.vector.tensor_add(
                            qk_rot[:, o * S + sb * 512: o * S + sb * 512 + 512],
                            t1, jp)
                    else:
                        nc.scalar.activation(
                            out=vt_sb[:, sb * 512:(sb + 1) * 512], in_=psl[sb],
                            func=mybir.ActivationFunctionType.Copy)
            # V: T-layout -> natural via PE transpose
            for t in range(NST):
                tp = jps.tile([128, 128], BF16, tag="jps")
                nc.tensor.transpose(tp, vt_sb[:, t * 128:(t + 1) * 128], id_sb)
                nc.vector.tensor_copy(v_nat[:, t * 128:(t + 1) * 128], tp)

        # ---------------- phase 2: attention ----------------
        aotp = outer.enter_context(tc.tile_pool(name="aot", bufs=1))
        wotp = outer.enter_context(tc.tile_pool(name="wotsb", bufs=1))
        # aot[d, j*S + s] = head j attention out (normalized), T-layout
        aot = aotp.tile([128, NJT * S], BF16)
        wot_sb = wotp.tile([128, NJT * D], BF16)

        with ExitStack() as ph2:
            etp = ph2.enter_context(tc.tile_pool(name="expt", bufs=6))
            rbp = ph2.enter_context(tc.tile_pool(name="rbc", bufs=2))
            spsp = ph2.enter_context(tc.tile_pool(name="sps", bufs=4, space="PSUM"))
            outpp = ph2.enter_context(tc.tile_pool(name="outps", bufs=2, space="PSUM"))
            rpsp = ph2.enter_context(tc.tile_pool(name="rps", bufs=2, space="PSUM"))

            for j in range(NJT):
                for half in range(2):
                    hw_ = D // 2
                    nc.sync.dma_start(
                        out=wot_sb[:, j * D + half * hw_: j * D + (half + 1) * hw_],
                        in_=wot_d[j, :, half * hw_:(half + 1) * hw_])

            for jq in range(NSB):
                nk = 4 * jq + 4       # causal: k-tiles 0..4jq+3
                for h in range(HQ):
                    outps = outpp.tile([128, 512], F32, tag="outps")
                    rps = rpsp.tile([128, 512], F32, tag="rps")
                    for kt in range(nk):
                        delta = kt - 4 * jq
                        a = max(delta, 0) * 128   # live q range [a, 512)
                        sps = spsp.tile([128, 512], F32, tag="sps")
                        nc.tensor.matmul(
                            sps[:, a:],
                            qk_rot[:, HQ * S + kt * 128: HQ * S + (kt + 1) * 128],
                            qk_rot[:, h * S + jq * 512 + a: h * S + jq * 512 + 512],
                            start=True, stop=True)
                        if delta >= 0:
                            # block-diagonal subtile: additive causal mask
                            nc.vector.tensor_add(
                                sps[:, a:a + 128], sps[:, a:a + 128], mask_sb)
                        et = etp.tile([128, 512], BF16, tag="et")
                        nc.scalar.activation(
                            out=et[:, a:], in_=sps[:, a:],
                            func=mybir.ActivationFunctionType.Exp, scale=SCALE)
                        nc.tensor.matmul(
                            outps[:, a:], v_nat[:, kt * 128:(kt + 1) * 128],
                            et[:, a:],
                            start=(kt == 0), stop=(kt == nk - 1))
                        # all-ones stationary -> denominators land
                        # partition-broadcast: rps[m, q] = r[q] for every m
                        nc.tensor.matmul(
                            rps[:, a:], ones_sb, et[:, a:],
                            start=(kt == 0), stop=(kt == nk - 1))
                    rinv = rbp.tile([128, 512], F32, tag="rinv")
                    nc.vector.reciprocal_approx_fast(out=rinv, in_=rps)
                    nc.vector.tensor_mul(
                        aot[:, h * S + jq * 512: h * S + jq * 512 + 512],
                        outps, rinv)

        # ---------------- phase 3: output projection ----------------
        with ExitStack() as ph3:
            stg = ph3.enter_context(tc.tile_pool(name="stage", bufs=10))
            opsp = ph3.enter_context(tc.tile_pool(name="ops", bufs=8, space="PSUM"))

            for stc in range(NST):
                psl = [opsp.tile([128, 512], F32, tag="ops",
                                 name=f"ops_{stc}_{i}")
                       for i in range(NEB)]
                for j in range(NJT):
                    for eb in range(NEB):
                        nc.tensor.matmul(
                            psl[eb],
                            aot[:, j * S + stc * 128: j * S + (stc + 1) * 128],
                            wot_sb[:, j * D + eb * 512: j * D + eb * 512 + 512],
                            start=(j == 0), stop=(j == NJT - 1))
                for eb in range(NEB):
                    stage = stg.tile([128, 512], BF16, tag="stage")
                    nc.scalar.activation(
                        out=stage, in_=psl[eb],
                        func=mybir.ActivationFunctionType.Copy)
                    # last row-chunk: halve the store DMAs so the kernel
                    # tail is not one full 256KB transfer deep
                    nsp = 2 if stc == NST - 1 else 1
                    for sp in range(nsp):
                        w_ = 512 // nsp
                        nc.sync.dma_start(
                            out=out_d[stc * 128:(stc + 1) * 128,
                                      eb * 512 + sp * w_:
                                      eb * 512 + (sp + 1) * w_],
                            in_=stage[:, sp * w_:(sp + 1) * w_])

    nc.compile()
    return nc


# ---------------------------------------------------------------------------
# host-side prep


def make_consts(cos, sin):
    """cos/sin: [S, 64] f32 -> replicated T-layout + J + identity + diag mask."""
    cos2 = np.repeat(np.ascontiguousarray(cos.T), 2, axis=0).astype(np.float32)
    sin2 = np.repeat(np.ascontiguousarray(sin.T), 2, axis=0).astype(np.float32)
    J = np.zeros((128, 128), np.float32)
    for p in range(64):
        J[2 * p, 2 * p + 1] = -1.0
        J[2 * p + 1, 2 * p] = 1.0
    jt = np.ascontiguousarray(J.T).astype(NBF)
    ident = np.eye(128, dtype=NBF)
    k_idx = np.arange(128)[:, None]
    q_idx = np.arange(128)[None, :]
    maskt = np.where(q_idx >= k_idx, 0.0, NEG).astype(np.float32)  # [k, q]
    return cos2, sin2, jt, ident, maskt.astype(NBF)


def prep_all(x, wq, wk, wv, wo, cos, sin, n_cores=N_CORES):
    NCT = D // 128
    x2 = np.asarray(x, np.float32).reshape(S, D)
    xt = np.ascontiguousarray(x2.T).astype(NBF)
    wq = np.asarray(wq, np.float32)
    wk = np.asarray(wk, np.float32)
    wv = np.asarray(wv, np.float32)
    wo = np.asarray(wo, np.float32)
    cos2, sin2, jt, ident, maskt = make_consts(
        np.asarray(cos, np.float32), np.asarray(sin, np.float32))
    in_maps = []
    for g in range(n_cores):
        w_cat = np.concatenate(
            [wq[g * 512:(g + 1) * 512],
             wk[g * 128:(g + 1) * 128],
             wv[g * 128:(g + 1) * 128]], axis=0)          # [768, D]
        # wt[o, p, c*128 + f] = w_cat[o*128 + f, c*128 + p]
        wt = np.ascontiguousarray(
            w_cat.reshape(6, 128, NCT, 128).transpose(0, 3, 2, 1)
        ).reshape(6, 128, NCT * 128).astype(NBF)
        wot = np.ascontiguousarray(
            wo[:, g * 512:(g + 1) * 512].T).reshape(4, 128, D).astype(NBF)
        in_maps.append({
            "xt": xt, "wt": wt, "wot": wot, "cos2": cos2, "sin2": sin2,
            "jt": jt, "ident": ident, "maskt": maskt,
        })
    return in_maps


_NC_CACHE = None


def _get_nc():
    global _NC_CACHE
    if _NC_CACHE is None:
        _NC_CACHE = build_nc()
    return _NC_CACHE


def kernel(x, wq, wk, wv, wo, cos, sin, mask, start_pos):
    # mask is the standard causal mask (start_pos=0 prefill) — the kernel
    # applies causality structurally, so neither input is shipped.
    from concourse.bass_utils import run_bass_kernel_spmd

    nc = _get_nc()
    in_maps = prep_all(x, wq, wk, wv, wo, cos, sin)
    res = run_bass_kernel_spmd(nc, in_maps, core_ids=list(range(N_CORES)))
    acc = np.zeros((S, D), np.float32)
    for r in res.results:
        acc += r["out"].astype(np.float32)
    return acc.reshape(1, S, D)

